# revision 3
# baseline (speedup 1.0000x reference)
"""Trainium2 kernel for nn_LinearVectorTransform (MoE-style routed bmv).

Reference computation:  pred[b, j] = sum_i before[b, i] * weights[action[b], i, j]
with B=1024 samples, V=768, A=8 expert matrices.

Sharding strategy (expert-parallel, chosen over the data-parallel hint):
core `a` owns expert `a`'s [768, 768] weight matrix and processes exactly the
samples routed to it. Each weight matrix is then read from HBM exactly once
across the whole chip (18.9 MB total) instead of 8 times under data-parallel
replication — an 8x cut in the dominant HBM traffic for this memory-bound
problem. The routing/dispatch (grouping sample rows by action) happens on the
host as part of sharding, exactly like an MoE a2a dispatch; the O(B*V^2)
compute runs on device as one dense matmul per core.

Per-core device kernel computes out.T = W.T-chunks (stationary) applied to
xT (moving):  psum[j, c] += sum_k w_sb[k][:, j]^T @ x_sb[k][:, c].
Streaming X as the moving operand makes PE time proportional to the actual
routed-batch capacity (~144 rows) instead of the padded 768 weight columns,
and loads every [128,128] weight chunk into the PE array exactly once.
"""

import numpy as np
from functools import lru_cache

B = 1024          # batch
V = 768           # vec size
A = 8             # experts == cores
N_CORES = 8
P = 128           # partitions
K_TILES = V // P  # 6 contraction tiles
J_TILES = V // P  # 6 output-column strips (rows of out.T)
DEF_CAP = 144     # per-expert routed-row capacity (seed-0 max count is 142;
                  # Binomial(1024, 1/8) mean 128, sd ~10.6). Recompiled larger
                  # if ever exceeded.
MAX_MOVING = 512  # fp32 moving-operand limit per matmul
MM_DTYPE = "float32"


def _ceil_to(x: int, m: int) -> int:
    return -(-x // m) * m


@lru_cache(maxsize=4)
def _compiled(cap: int, mm_dtype: str):
    import concourse.bacc as bacc
    import concourse.mybir as mybir
    import concourse.tile as tile

    f32 = mybir.dt.float32
    mm_dt = getattr(mybir.dt, mm_dtype)

    # Moving-dim chunks of <= 512 (single chunk for the typical cap ~144).
    n_chunks = -(-cap // MAX_MOVING)
    chunks = []
    off = 0
    for i in range(n_chunks):
        sz = min(MAX_MOVING, cap - off)
        chunks.append((off, sz))
        off += sz

    nc = bacc.Bacc("TRN2", target_bir_lowering=False, debug=False)
    xT = nc.dram_tensor("xt", [V, cap], f32, kind="ExternalInput").ap()
    w = nc.dram_tensor("w", [V, V], f32, kind="ExternalInput").ap()
    # Output stored transposed: out_t[j, c] = pred[row c, col j].
    out_t = nc.dram_tensor("out_t", [V, cap], f32, kind="ExternalOutput").ap()

    with tile.TileContext(nc) as tc:
        with (
            tc.tile_pool(name="io", bufs=1) as io_pool,
            tc.tile_pool(name="ps", bufs=1, space="PSUM") as ps_pool,
        ):
            # Activations first (gate the first matmul), then weights per
            # K-tile so matmuls start as soon as each 384 KB slab lands.
            x_sb = io_pool.tile([P, K_TILES, cap], f32, tag="x", name="x")
            nc.sync.dma_start(x_sb[:], xT.rearrange("(k p) c -> p k c", p=P))
            w_sb = []
            for k in range(K_TILES):
                wk = io_pool.tile([P, V], f32, tag=f"w{k}", name=f"w{k}")
                nc.sync.dma_start(wk[:], w[k * P:(k + 1) * P, :])
                w_sb.append(wk)

            ps = [
                ps_pool.tile([P, cap], f32, tag=f"ps{j}", name=f"ps{j}")
                for j in range(J_TILES)
            ]
            # K-outer streaming: each weight slab is consumed right after its
            # DMA lands; all 6 j-strip PSUM banks accumulate in parallel.
            for k in range(K_TILES):
                for j in range(J_TILES):
                    for off, sz in chunks:
                        nc.tensor.matmul(
                            ps[j][:, off:off + sz],
                            w_sb[k][:, j * P:(j + 1) * P].bitcast(mm_dt),
                            x_sb[:, k, off:off + sz].bitcast(mm_dt),
                            start=(k == 0),
                            stop=(k == K_TILES - 1),
                        )
            ot_sb = io_pool.tile([P, J_TILES, cap], f32, tag="ot", name="ot")
            for j in range(J_TILES):
                nc.vector.tensor_copy(ot_sb[:, j, :], ps[j][:])
            nc.sync.dma_start(out_t.rearrange("(j p) c -> p j c", p=P), ot_sb[:])

    nc.compile()
    return nc


def kernel(before: np.ndarray, action: np.ndarray, weights: np.ndarray) -> np.ndarray:
    from concourse.bass_utils import run_bass_kernel_spmd

    before = np.ascontiguousarray(np.asarray(before), dtype=np.float32)
    weights = np.ascontiguousarray(np.asarray(weights), dtype=np.float32)
    acts = np.asarray(action).astype(np.int64)
    n_rows, vec = before.shape
    assert vec == V and weights.shape == (A, V, V)

    idx = [np.flatnonzero(acts == a) for a in range(A)]
    max_count = max(len(i) for i in idx)
    cap = DEF_CAP if max_count <= DEF_CAP else _ceil_to(max_count, 16)

    nc = _compiled(cap, MM_DTYPE)

    in_maps = []
    for a in range(A):
        xTa = np.zeros((V, cap), dtype=np.float32)
        if len(idx[a]):
            xTa[:, :len(idx[a])] = before[idx[a]].T
        in_maps.append({"xt": xTa, "w": weights[a]})

    res = run_bass_kernel_spmd(nc, in_maps, core_ids=list(range(N_CORES)))

    out = np.empty((n_rows, V), dtype=np.float32)
    for a in range(A):
        if len(idx[a]):
            out[idx[a]] = res.results[a]["out_t"].T[:len(idx[a])]
    return out
